# revision 8
# baseline (speedup 1.0000x reference)
"""AraBERT BiLSTM-CRF NLL loss on 8 TRN2 NeuronCores.

Strategy (data-parallel, hint-conformant): batch 32 sharded 4-per-core; each
core runs a fused BiLSTM (bf16 matmuls, fp32 cell state), projection to
emissions, and a chunk-parallel linear-space CRF partition-function scan
(matrix-product reformulation, scale 1/15 folded into exp(trans)); tiny
per-core partials (em-path score accumulators, CRF chunk products) are
DMA'd out and combined on host (the "all-reduce the scalar" step).

Numerics: tanh computed via sigmoid (x2 folded into weights); h stored as
h/2 (x2 folded into Whh/Wp); CRF runs in linear space with all logs taken
on host, deterministic (S-1)*log(15) rescale correction.
"""
import sys

sys.path.insert(0, "/opt/trn_rl_repo")

import numpy as np
import ml_dtypes

import concourse.bass as bass
import concourse.mybir as mybir
from concourse.bass_utils import run_bass_kernel_spmd
from concourse.tile import TileContext
from concourse.vector_clock import ScopedClock

# ---------------------------------------------------------------------------
# Workaround: this walrus build rejects a Drain instruction carrying more than
# one sync wait (TPB_CTRL_NO_STRUCT).  TileContext's tail drain aggregates one
# wait per outstanding proc; split them across single-wait NOPs.
# ---------------------------------------------------------------------------


def _patched_drain_and_barrier(self, tick_clock, wait_clock):
    nc = self.nc
    probe = nc.sync.nop(hint="tail_wait_probe", nofuse=True)
    wait_clock.add_sem_waits(probe.ins, ScopedClock({None: tick_clock.global_clock}))
    waits = list(probe.ins.sync_info.on_wait or []) if probe.ins.sync_info else []
    if len(waits) > 1:
        probe.ins.sync_info.on_wait = waits[:1]
        for w in waits[1:]:
            n = nc.sync.nop(hint="tail_wait_split", nofuse=True)
            n.ins.sync_info = mybir.SyncInfo(on_wait=[w], on_update=[])
    nc.sync.drain()
    nc.all_engine_barrier()
    assert self.sems is not None
    popped = nc._tile_sem_poison_stack.pop()
    assert popped is self._sem_poison
    nc.clear_and_free_semaphores(list(self.sems.allocated().values()))
    nc.all_engine_barrier()


TileContext._drain_and_barrier = _patched_drain_and_barrier


# Walrus in this container accepts only ONE sync wait per instruction for
# several instruction classes.  After Tile scheduling, split any instruction
# carrying N>1 waits: the first N-1 waits move to same-engine NOPs inserted
# immediately before it (program order on the engine preserves semantics).
_MAXW = 1


def _split_multi_waits(nc):
    n_split = 0
    for bbname, bbwrap in nc.bb_map.items():
        bb = bbwrap.bb
        il = bb.instructions
        i = 0
        while i < len(il):
            inst = il[i]
            si = inst.sync_info
            if si is not None and si.on_wait and len(si.on_wait) > _MAXW:
                waits = list(si.on_wait)
                si.on_wait = waits[-_MAXW:]
                pre = waits[:-_MAXW]
                for k, w in enumerate(pre):
                    nop = mybir.InstNoOp(
                        name=f"{inst.name}_w{k}",
                        sync_info=mybir.SyncInfo(on_wait=[w], on_update=[]),
                        bass_nofuse=True,
                        engine=inst.engine,
                    )
                    il.insert(i, nop)
                    i += 1
                n_split += 1
            i += 1
    return n_split

# ---------------------------------------------------------------------------

B, S, E, H, T = 32, 512, 768, 128, 15
NCORES = 8
BL = B // NCORES          # 4 sequences per core
SB = S * BL               # token columns per core (pos-major: col = pos*BL + b)
F32, F32R, BF16 = mybir.dt.float32, mybir.dt.float32r, mybir.dt.bfloat16
AF = mybir.ActivationFunctionType
ALU = mybir.AluOpType
CRF_SLOTS = 16            # CRF chunk length (scan slots)
bf16 = ml_dtypes.bfloat16


def _crf_chunks(s):
    return -(-(s - 1) // CRF_SLOTS)  # ceil((S-1)/16)


def build_nc(s=S):
    sb = s * BL
    ncw = min(512, sb)            # matmul N-chunk width
    nch = sb // ncw               # N-chunks per (dir, gate)
    cc = _crf_chunks(s)           # CRF chunks
    nc = bass.Bass("TRN2", target_bir_lowering=False, debug=False, num_devices=NCORES)

    xt = nc.dram_tensor("xt", [E, sb], BF16, kind="ExternalInput").ap()
    wih = nc.dram_tensor("wih", [E, 8 * H], BF16, kind="ExternalInput").ap()
    whh = nc.dram_tensor("whh", [H, 8 * H], BF16, kind="ExternalInput").ap()
    bia = nc.dram_tensor("bia", [H, 8], F32, kind="ExternalInput").ap()
    gmat = nc.dram_tensor("gmat", [2 * H, sb], BF16, kind="ExternalInput").ap()
    wpt = nc.dram_tensor("wpt", [2 * H, T], BF16, kind="ExternalInput").ap()
    bp15 = nc.dram_tensor("bp15", [T, 1], F32, kind="ExternalInput").ap()
    pp = nc.dram_tensor("pp", [T, T], BF16, kind="ExternalInput").ap()
    u0 = nc.dram_tensor("u0", [T, cc * 60], BF16, kind="ExternalInput").ap()

    out_acc = nc.dram_tensor("out_acc", [H, 2], F32, kind="ExternalOutput").ap()
    out_u = nc.dram_tensor("out_u", [T, cc * 60], BF16, kind="ExternalOutput").ap()
    out_e0 = nc.dram_tensor("out_e0", [T, BL], F32, kind="ExternalOutput").ap()

    with TileContext(nc) as tc:
        with (
            tc.tile_pool(name="static", bufs=1) as sp,
            tc.tile_pool(name="pzrec", bufs=2, space="PSUM") as pzrec,
        ):
            # ---- static SBUF tiles ----
            xt_sb = sp.tile([128, 6, sb], BF16, tag="xt")
            wih_sb = sp.tile([128, 6, 8 * H], BF16, tag="wih")
            whh_sb = sp.tile([128, 2, 4, H], BF16, tag="whh")
            bia_sb = sp.tile([128, 8], F32, tag="bia")
            g_sb = sp.tile([128, 2, sb], BF16, tag="g")
            wp_sb = sp.tile([128, 2, T], BF16, tag="wp")
            bp_sb = sp.tile([T, 1], F32, tag="bp")
            pp_sb = sp.tile([T, T], BF16, tag="pp")
            u_sb = sp.tile([T, cc * 60], BF16, tag="u")
            zx0 = sp.tile([128, 4, sb], F32, tag="zx0")
            zx1 = sp.tile([128, 4, sb], F32, tag="zx1")
            zx = [zx0, zx1]
            hh = sp.tile([128, 2, s, BL], BF16, tag="hh")
            e_sb = sp.tile([128, sb + 128], F32, tag="e")
            zsb = sp.tile([128, 2, 4, BL], F32, tag="z")
            sg = sp.tile([128, 2, 4, BL], F32, tag="sg")
            vv = sp.tile([128, 2, BL], F32, tag="vv")
            tt = sp.tile([128, 2, BL], F32, tag="tt")
            c2 = sp.tile([128, 2, BL], F32, tag="c2")
            sc = sp.tile([128, 2, BL], F32, tag="sc")
            zero4 = sp.tile([128, BL], BF16, tag="zero4")
            acc_sb = sp.tile([128, 2], F32, tag="acc")

            # ---- input DMAs (SP engine) ----
            for k in range(6):
                nc.sync.dma_start(out=xt_sb[:, k, :], in_=xt[k * 128:(k + 1) * 128, :])
                nc.sync.dma_start(out=wih_sb[:, k, :], in_=wih[k * 128:(k + 1) * 128, :])
            nc.sync.dma_start(
                out=whh_sb[:, :, :, :],
                in_=whh.rearrange("k (d g j) -> k d g j", d=2, g=4),
            )
            nc.sync.dma_start(out=bia_sb[:, :], in_=bia[:, :])
            for d in range(2):
                nc.sync.dma_start(out=g_sb[:, d, :], in_=gmat[d * 128:(d + 1) * 128, :])
                nc.sync.dma_start(out=wp_sb[:, d, :], in_=wpt[d * 128:(d + 1) * 128, :])
            nc.sync.dma_start(out=bp_sb[:, :], in_=bp15[:, :])
            nc.sync.dma_start(out=pp_sb[:, :], in_=pp[:, :])
            nc.sync.dma_start(out=u_sb[:, :], in_=u0[:, :])
            nc.vector.memset(zero4[:, :], 0.0)
            nc.vector.memset(c2[:, :, :], 0.0)

            # ---- Zx precompute groups: one (dir, n-chunk, gate) at a time ----
            def emit_zx_group(pool, d, n, g):
                ps = pool.tile([128, ncw], F32, tag="pzx")
                for k in range(6):
                    nc.tensor.matmul(
                        ps[:, :],
                        lhsT=wih_sb[:, k, d * 512 + g * 128:d * 512 + (g + 1) * 128],
                        rhs=xt_sb[:, k, n * ncw:(n + 1) * ncw],
                        start=(k == 0), stop=(k == 5),
                    )
                nc.scalar.activation(
                    zx[d][:, g, n * ncw:(n + 1) * ncw], ps[:, :],
                    AF.Identity, bias=bia_sb[:, d * 4 + g:d * 4 + g + 1], scale=1.0,
                )

            # preamble chunks: fwd needs n=0 first; bwd consumes from the end
            first = [(0, 0), (1, nch - 1)]
            rest = []
            for k in range(1, nch):
                rest.append((0, k))
                rest.append((1, nch - 1 - k))
            rest_groups = [(d, n, g) for (d, n) in rest for g in range(4)]

            def h_rhs(d, t):
                if t == 0:
                    return zero4[:, :]
                p_prev = (t - 1) if d == 0 else (s - t)
                return hh[:, d, p_prev, :]

            with tc.tile_pool(name="pzx", bufs=2, space="PSUM") as pzx:
                for (d, n) in first:
                    for g in range(4):
                        emit_zx_group(pzx, d, n, g)

                gi = 0
                for t in range(s):
                    if t >= 8 and (t - 8) % 12 == 0 and gi < len(rest_groups):
                        d_, n_, g_ = rest_groups[gi]
                        emit_zx_group(pzx, d_, n_, g_)
                        gi += 1
                    # recurrent matmuls + z assemble per dir
                    for d in range(2):
                        pos = t if d == 0 else (s - 1 - t)
                        ps = pzrec.tile([128, 4, BL], F32, tag=f"pz{d}")
                        for g in range(4):
                            nc.tensor.matmul(
                                ps[:, g, :],
                                lhsT=whh_sb[:, d, g, :],
                                rhs=h_rhs(d, t),
                                start=(g == 0), stop=(g == 3),
                            )
                        nc.vector.tensor_tensor(
                            zsb[:, d], ps[:, :, :],
                            zx[d][:, :, pos * BL:(pos + 1) * BL], ALU.add,
                        )
                    nc.scalar.activation(sg[:, :, :, :], zsb[:, :, :, :], AF.Sigmoid)
                    nc.vector.scalar_tensor_tensor(
                        vv[:, :, :], sg[:, :, 2, :], 0.5, sg[:, :, 0, :],
                        op0=ALU.subtract, op1=ALU.mult,
                    )
                    nc.gpsimd.tensor_tensor(
                        tt[:, :, :], sg[:, :, 1, :], c2[:, :, :], ALU.mult,
                    )
                    nc.vector.scalar_tensor_tensor(
                        c2[:, :, :], vv[:, :, :], 4.0, tt[:, :, :],
                        op0=ALU.mult, op1=ALU.add,
                    )
                    nc.scalar.activation(sc[:, :, :], c2[:, :, :], AF.Sigmoid)
                    for d in range(2):
                        pos = t if d == 0 else (s - 1 - t)
                        nc.vector.scalar_tensor_tensor(
                            hh[:, d, pos, :], sc[:, d, :], 0.5, sg[:, d, 3, :],
                            op0=ALU.subtract, op1=ALU.mult,
                        )
                assert gi == len(rest_groups)

            # ---- projection -> emissions -> E = exp(em + bp) ----
            with tc.tile_pool(name="pproj", bufs=2, space="PSUM") as pproj:
                for n in range(nch):
                    ps = pproj.tile([T, ncw], F32, tag="pp")
                    for d in range(2):
                        nc.tensor.matmul(
                            ps[:, :], lhsT=wp_sb[:, d, :],
                            rhs=hh[:, d].rearrange("p s b -> p (s b)")[:, n * ncw:(n + 1) * ncw],
                            start=(d == 0), stop=(d == 1),
                        )
                    nc.scalar.activation(
                        e_sb[0:T, n * ncw:(n + 1) * ncw], ps[:, :],
                        AF.Exp, bias=bp_sb[:, :], scale=1.0,
                    )

            # ---- CRF chunk-parallel scan: U <- diag(E_t) @ (P'^T U) ----
            with tc.tile_pool(name="pcrf", bufs=2, space="PSUM") as pcrf:
                for tau in range(CRF_SLOTS):
                    nact = sum(
                        1 for c in range(cc) if c * CRF_SLOTS + tau + 1 <= s - 1
                    )
                    if nact == 0:
                        continue
                    c0 = 0
                    while c0 < nact:
                        cw = min(8, nact - c0)
                        ps = pcrf.tile([T, 480], F32, tag="pc")
                        nc.tensor.matmul(
                            ps[:, 0:cw * 60], lhsT=pp_sb[:, :],
                            rhs=u_sb[:, c0 * 60:(c0 + cw) * 60],
                            start=True, stop=True,
                        )
                        ebase = (tau + 1 + c0 * CRF_SLOTS) * BL
                        eap = (
                            e_sb[0:T, ebase:ebase + CRF_SLOTS * BL * cw]
                            .rearrange("p (c y) -> p c y", c=cw)[:, :, 0:BL]
                            .unsqueeze(2)
                            .broadcast_to((T, cw, T, BL))
                        )
                        nc.vector.tensor_tensor(
                            u_sb[:, c0 * 60:(c0 + cw) * 60]
                            .rearrange("p (c m b) -> p c m b", c=cw, m=T),
                            ps[:, 0:cw * 60]
                            .rearrange("p (c m b) -> p c m b", c=cw, m=T),
                            eap, ALU.mult,
                        )
                        c0 += cw

            # ---- emission-path gold score accumulators ----
            for d in range(2):
                scratch = sp.tile([128, sb], F32, tag="scr")
                nc.vector.scalar_tensor_tensor(
                    scratch[:, :],
                    hh[:, d].rearrange("p s b -> p (s b)"), 1.0, g_sb[:, d, :],
                    op0=ALU.mult, op1=ALU.mult,
                    accum_out=acc_sb[:, d:d + 1],
                )

            # ---- outputs ----
            nc.sync.dma_start(out=out_acc[:, :], in_=acc_sb[:, :])
            nc.sync.dma_start(out=out_u[:, :], in_=u_sb[:, :])
            nc.sync.dma_start(out=out_e0[:, :], in_=e_sb[0:T, 0:BL])
    return nc


# ---------------------------------------------------------------------------
# Host side
# ---------------------------------------------------------------------------

_NC_CACHE = {}


def _get_nc(s):
    if s not in _NC_CACHE:
        _NC_CACHE[s] = build_nc(s)
    return _NC_CACHE[s]


def kernel(x, tags, mask, Wih_f, Whh_f, bih_f, bhh_f, Wih_b, Whh_b, bih_b, bhh_b,
           Wp, bp, trans, start_t, end_t):
    x = np.asarray(x, np.float32)
    tags = np.asarray(tags)
    mask = np.asarray(mask)
    assert mask.all(), "kernel assumes mask == ones (spec fill: ones)"
    b, s, e = x.shape
    assert (b, e) == (B, E)
    cc = _crf_chunks(s)

    Wih = {0: np.asarray(Wih_f, np.float64), 1: np.asarray(Wih_b, np.float64)}
    Whh = {0: np.asarray(Whh_f, np.float64), 1: np.asarray(Whh_b, np.float64)}
    bias = {
        0: np.asarray(bih_f, np.float64) + np.asarray(bhh_f, np.float64),
        1: np.asarray(bih_b, np.float64) + np.asarray(bhh_b, np.float64),
    }
    Wp64 = np.asarray(Wp, np.float64)
    bp64 = np.asarray(bp, np.float64)
    trans64 = np.asarray(trans, np.float64)
    start64 = np.asarray(start_t, np.float64)
    end64 = np.asarray(end_t, np.float64)

    # gate folds: g-gate rows x2 (tanh via sigmoid); Whh/Wp x2 (h stored as h/2)
    gsl = slice(2 * H, 3 * H)
    wih_cols, whh_cols, bia_cols = [], [], []
    for d in range(2):
        wi = Wih[d].copy(); wi[gsl] *= 2.0
        wh = 2.0 * Whh[d].copy(); wh[gsl] *= 2.0
        bi = bias[d].copy(); bi[gsl] *= 2.0
        wih_cols.append(wi.T)        # (E, 4H)
        whh_cols.append(wh.T)        # (H, 4H)
        bia_cols.append(bi.reshape(4, H).T)   # (H, 4)
    wih_host = np.concatenate(wih_cols, axis=1).astype(bf16)       # (E, 8H)
    whh_host = np.concatenate(whh_cols, axis=1).astype(bf16)       # (H, 8H)
    bia_host = np.concatenate(bia_cols, axis=1).astype(np.float32)  # (H, 8)
    Wp_eff = 2.0 * Wp64                                             # (T, 2H)
    wpt_host = Wp_eff.T.astype(bf16)                                # (2H, T)
    bp_host = bp64.reshape(T, 1).astype(np.float32)
    pp_host = (np.exp(trans64) / 15.0).astype(bf16)           # (T, T)
    u0_host = np.zeros((T, cc * 60), bf16)
    for c in range(cc):
        for seq in range(BL):
            for m in range(T):
                u0_host[m, c * 60 + m * BL + seq] = 1.0

    in_maps = []
    for core in range(NCORES):
        bsl = slice(core * BL, (core + 1) * BL)
        xs = x[bsl]                                  # (BL, s, E)
        xt_host = np.ascontiguousarray(
            xs.transpose(2, 1, 0).reshape(E, s * BL)
        ).astype(bf16)                               # col = pos*BL + b
        tg = tags[bsl]                               # (BL, s)
        gm = np.empty((2 * H, s * BL), np.float64)
        gathered = Wp_eff[tg.T.reshape(-1)]          # (s*BL, 2H) rows pos-major
        gm[:, :] = gathered.T
        in_maps.append({
            "xt": xt_host,
            "wih": wih_host, "whh": whh_host, "bia": bia_host,
            "gmat": gm.astype(bf16), "wpt": wpt_host, "bp15": bp_host,
            "pp": pp_host, "u0": u0_host,
        })

    nc = _get_nc(s)
    runner = globals()["run_bass_kernel_spmd"]
    if not getattr(runner, "_is_sim", False) and not getattr(nc, "_waits_split", False):
        _split_multi_waits(nc)
        nc._waits_split = True
    res = runner(nc, in_maps, core_ids=list(range(NCORES)))

    # ---- host epilogue ----
    logS = np.log(15.0)
    exp_start = np.exp(start64)
    exp_end = np.exp(end64)
    total = 0.0
    for core in range(NCORES):
        r = res.results[core]
        acc = np.asarray(r["out_acc"], np.float64)
        uu = np.asarray(r["out_u"], np.float64)
        e0 = np.asarray(r["out_e0"], np.float64)
        emscore = acc.sum()
        bsl = slice(core * BL, (core + 1) * BL)
        tg = tags[bsl]
        for seq in range(BL):
            tgq = tg[seq]
            score = (start64[tgq[0]] + trans64[tgq[:-1], tgq[1:]].sum()
                     + end64[tgq[-1]] + bp64[tgq].sum())
            rvec = exp_start * e0[:, seq]
            lz = 0.0
            for c in range(cc):
                ub = uu[:, c * 60 + seq:(c + 1) * 60:BL]   # (T, T): [j, m]
                rvec = ub @ rvec
                m = rvec.max()
                rvec /= m
                lz += np.log(m)
            lz += np.log(rvec @ exp_end) + (s - 1) * logS
            total += lz - score
        total -= emscore
    return np.asarray(total, np.float32)


# revision 10
# speedup vs baseline: 1.3388x; 1.3388x over previous
"""AraBERT BiLSTM-CRF NLL loss on 8 TRN2 NeuronCores.

Strategy (data-parallel, hint-conformant): batch 32 sharded 4-per-core; each
core runs a fused BiLSTM (bf16 matmuls, fp32 cell state), projection to
emissions, and a chunk-parallel linear-space CRF partition-function scan
(matrix-product reformulation, scale 1/15 folded into exp(trans)); tiny
per-core partials (em-path score accumulators, CRF chunk products) are
DMA'd out and combined on host (the "all-reduce the scalar" step).

Numerics: tanh computed via sigmoid (x2 folded into weights); h stored as
h/2 (x2 folded into Whh/Wp); CRF runs in linear space with all logs taken
on host, deterministic (S-1)*log(15) rescale correction.
"""
import sys

sys.path.insert(0, "/opt/trn_rl_repo")

import numpy as np
import ml_dtypes

import concourse.bass as bass
import concourse.mybir as mybir
from concourse.bass_utils import run_bass_kernel_spmd
from concourse.tile import TileContext
from concourse.vector_clock import ScopedClock

# ---------------------------------------------------------------------------
# Workaround: this walrus build rejects a Drain instruction carrying more than
# one sync wait (TPB_CTRL_NO_STRUCT).  TileContext's tail drain aggregates one
# wait per outstanding proc; split them across single-wait NOPs.
# ---------------------------------------------------------------------------


def _patched_drain_and_barrier(self, tick_clock, wait_clock):
    nc = self.nc
    probe = nc.sync.nop(hint="tail_wait_probe", nofuse=True)
    wait_clock.add_sem_waits(probe.ins, ScopedClock({None: tick_clock.global_clock}))
    waits = list(probe.ins.sync_info.on_wait or []) if probe.ins.sync_info else []
    if len(waits) > 1:
        probe.ins.sync_info.on_wait = waits[:1]
        for w in waits[1:]:
            n = nc.sync.nop(hint="tail_wait_split", nofuse=True)
            n.ins.sync_info = mybir.SyncInfo(on_wait=[w], on_update=[])
    nc.sync.drain()
    nc.all_engine_barrier()
    assert self.sems is not None
    popped = nc._tile_sem_poison_stack.pop()
    assert popped is self._sem_poison
    nc.clear_and_free_semaphores(list(self.sems.allocated().values()))
    nc.all_engine_barrier()


TileContext._drain_and_barrier = _patched_drain_and_barrier


# Walrus in this container accepts only ONE sync wait per instruction for
# several instruction classes.  After Tile scheduling, split any instruction
# carrying N>1 waits: the first N-1 waits move to same-engine NOPs inserted
# immediately before it (program order on the engine preserves semantics).
_MAXW = 1


def _split_multi_waits(nc):
    n_split = 0
    for bbname, bbwrap in nc.bb_map.items():
        bb = bbwrap.bb
        il = bb.instructions
        i = 0
        while i < len(il):
            inst = il[i]
            si = inst.sync_info
            if si is not None and si.on_wait and len(si.on_wait) > _MAXW:
                waits = list(si.on_wait)
                si.on_wait = waits[-_MAXW:]
                pre = waits[:-_MAXW]
                for k, w in enumerate(pre):
                    nop = mybir.InstNoOp(
                        name=f"{inst.name}_w{k}",
                        sync_info=mybir.SyncInfo(on_wait=[w], on_update=[]),
                        bass_nofuse=True,
                        engine=inst.engine,
                    )
                    il.insert(i, nop)
                    i += 1
                n_split += 1
            i += 1
    return n_split

# ---------------------------------------------------------------------------

B, S, E, H, T = 32, 512, 768, 128, 15
NCORES = 8
BL = B // NCORES          # 4 sequences per core
SB = S * BL               # token columns per core (pos-major: col = pos*BL + b)
F32, F32R, BF16 = mybir.dt.float32, mybir.dt.float32r, mybir.dt.bfloat16
AF = mybir.ActivationFunctionType
ALU = mybir.AluOpType
CRF_SLOTS = 16            # CRF chunk length (scan slots)
bf16 = ml_dtypes.bfloat16


def _crf_chunks(s):
    return -(-(s - 1) // CRF_SLOTS)  # ceil((S-1)/16)


def build_nc(s=S):
    sb = s * BL
    ncw = min(512, sb)            # matmul N-chunk width
    nch = sb // ncw               # N-chunks per (dir, gate)
    cc = _crf_chunks(s)           # CRF chunks
    nc = bass.Bass("TRN2", target_bir_lowering=False, debug=False, num_devices=NCORES)

    xt = nc.dram_tensor("xt", [E, sb], BF16, kind="ExternalInput").ap()
    wih = nc.dram_tensor("wih", [E, 8 * H], BF16, kind="ExternalInput").ap()
    whh = nc.dram_tensor("whh", [H, 8 * H], BF16, kind="ExternalInput").ap()
    bia = nc.dram_tensor("bia", [H, 8], F32, kind="ExternalInput").ap()
    gmat = nc.dram_tensor("gmat", [2 * H, sb], BF16, kind="ExternalInput").ap()
    wpt = nc.dram_tensor("wpt", [2 * H, T], BF16, kind="ExternalInput").ap()
    bp15 = nc.dram_tensor("bp15", [T, 1], F32, kind="ExternalInput").ap()
    pp = nc.dram_tensor("pp", [T, T], BF16, kind="ExternalInput").ap()
    u0 = nc.dram_tensor("u0", [T, cc * 60], BF16, kind="ExternalInput").ap()
    ident = nc.dram_tensor("ident", [H, H], BF16, kind="ExternalInput").ap()

    out_acc = nc.dram_tensor("out_acc", [H, 2], F32, kind="ExternalOutput").ap()
    out_u = nc.dram_tensor("out_u", [T, cc * 60], BF16, kind="ExternalOutput").ap()
    out_e0 = nc.dram_tensor("out_e0", [T, BL], F32, kind="ExternalOutput").ap()

    with TileContext(nc) as tc:
        with (
            tc.tile_pool(name="static", bufs=1) as sp,
            tc.tile_pool(name="pzrec", bufs=2, space="PSUM") as pzrec,
        ):
            # ---- static SBUF tiles ----
            xt_sb = sp.tile([128, 6, sb], BF16, tag="xt")
            wih_sb = sp.tile([128, 6, 8 * H], BF16, tag="wih")
            whh_sb = sp.tile([128, 2, 4, H], BF16, tag="whh")
            bia_sb = sp.tile([128, 8], F32, tag="bia")
            g_sb = sp.tile([128, 2, sb], BF16, tag="g")
            wp_sb = sp.tile([128, 2, T], BF16, tag="wp")
            bp_sb = sp.tile([T, 1], F32, tag="bp")
            pp_sb = sp.tile([T, T], BF16, tag="pp")
            u_sb = sp.tile([T, cc * 60], BF16, tag="u")
            zx0 = sp.tile([128, 4, sb], BF16, tag="zx0")
            zx1 = sp.tile([128, 4, sb], BF16, tag="zx1")
            zx = [zx0, zx1]
            id_sb = sp.tile([128, H], BF16, tag="id_sb")
            hh = sp.tile([128, 2, s, BL], BF16, tag="hh")
            e_sb = sp.tile([128, sb + 128], F32, tag="e")
            sg_f = sp.tile([128, 4, BL], F32, tag="sg_f")
            sg_b = sp.tile([128, 4, BL], F32, tag="sg_b")
            vv_f = sp.tile([128, BL], F32, tag="vv_f")
            vv_b = sp.tile([128, BL], F32, tag="vv_b")
            tt_f = sp.tile([128, BL], F32, tag="tt_f")
            tt_b = sp.tile([128, BL], F32, tag="tt_b")
            c2_f = sp.tile([128, BL], F32, tag="c2_f")
            c2_b = sp.tile([128, BL], F32, tag="c2_b")
            sc_f = sp.tile([128, BL], F32, tag="sc_f")
            sc_b = sp.tile([128, BL], F32, tag="sc_b")
            sg = [sg_f, sg_b]; vv = [vv_f, vv_b]; tt = [tt_f, tt_b]
            c2 = [c2_f, c2_b]; sc = [sc_f, sc_b]
            zero4 = sp.tile([128, BL], BF16, tag="zero4")
            acc_sb = sp.tile([128, 2], F32, tag="acc")

            # ---- input DMAs (SP engine) ----
            for k in range(6):
                nc.sync.dma_start(out=xt_sb[:, k, :], in_=xt[k * 128:(k + 1) * 128, :])
                nc.sync.dma_start(out=wih_sb[:, k, :], in_=wih[k * 128:(k + 1) * 128, :])
            nc.sync.dma_start(
                out=whh_sb[:, :, :, :],
                in_=whh.rearrange("k (d g j) -> k d g j", d=2, g=4),
            )
            nc.sync.dma_start(out=bia_sb[:, :], in_=bia[:, :])
            for d in range(2):
                nc.sync.dma_start(out=g_sb[:, d, :], in_=gmat[d * 128:(d + 1) * 128, :])
                nc.sync.dma_start(out=wp_sb[:, d, :], in_=wpt[d * 128:(d + 1) * 128, :])
            nc.sync.dma_start(out=bp_sb[:, :], in_=bp15[:, :])
            nc.sync.dma_start(out=pp_sb[:, :], in_=pp[:, :])
            nc.sync.dma_start(out=u_sb[:, :], in_=u0[:, :])
            nc.sync.dma_start(out=id_sb[:, :], in_=ident[:, :])
            nc.vector.memset(zero4[:, :], 0.0)
            nc.vector.memset(c2_f[:, :], 0.0)
            nc.vector.memset(c2_b[:, :], 0.0)

            # ---- Zx precompute groups: one (dir, n-chunk, gate) at a time ----
            def emit_zx_group(pool, d, n, g):
                ps = pool.tile([128, ncw], F32, tag="pzx")
                for k in range(6):
                    nc.tensor.matmul(
                        ps[:, :],
                        lhsT=wih_sb[:, k, d * 512 + g * 128:d * 512 + (g + 1) * 128],
                        rhs=xt_sb[:, k, n * ncw:(n + 1) * ncw],
                        start=(k == 0), stop=(k == 5),
                    )
                nc.scalar.activation(
                    zx[d][:, g, n * ncw:(n + 1) * ncw], ps[:, :],
                    AF.Identity, bias=bia_sb[:, d * 4 + g:d * 4 + g + 1], scale=1.0,
                )

            # preamble chunks: fwd needs n=0 first; bwd consumes from the end
            first = [(0, 0), (1, nch - 1)]
            rest = []
            for k in range(1, nch):
                rest.append((0, k))
                rest.append((1, nch - 1 - k))
            rest_groups = [(d, n, g) for (d, n) in rest for g in range(4)]

            def h_rhs(d, t):
                if t == 0:
                    return zero4[:, :]
                p_prev = (t - 1) if d == 0 else (s - t)
                return hh[:, d, p_prev, :]

            with tc.tile_pool(name="pzx", bufs=2, space="PSUM") as pzx:
                for (d, n) in first:
                    for g in range(4):
                        emit_zx_group(pzx, d, n, g)

                gi = 0
                for t in range(s):
                    if t >= 8 and (t - 8) % 12 == 0 and gi < len(rest_groups):
                        d_, n_, g_ = rest_groups[gi]
                        emit_zx_group(pzx, d_, n_, g_)
                        gi += 1
                    # two independent per-direction chains
                    for d in range(2):
                        pos = t if d == 0 else (s - 1 - t)
                        ps = pzrec.tile([128, 4, BL], F32, tag=f"pz{d}")
                        nc.tensor.matmul(
                            ps[:, :, :], lhsT=id_sb[:, :],
                            rhs=zx[d][:, :, pos * BL:(pos + 1) * BL],
                            start=True, stop=False,
                        )
                        for g in range(4):
                            nc.tensor.matmul(
                                ps[:, g, :],
                                lhsT=whh_sb[:, d, g, :],
                                rhs=h_rhs(d, t),
                                start=False, stop=(g == 3),
                            )
                        nc.scalar.activation(sg[d][:, :, :], ps[:, :, :], AF.Sigmoid)
                        nc.vector.scalar_tensor_tensor(
                            vv[d][:, :], sg[d][:, 2, :], 0.5, sg[d][:, 0, :],
                            op0=ALU.subtract, op1=ALU.mult,
                        )
                        nc.gpsimd.tensor_tensor(
                            tt[d][:, :], sg[d][:, 1, :], c2[d][:, :], ALU.mult,
                        )
                        nc.vector.scalar_tensor_tensor(
                            c2[d][:, :], vv[d][:, :], 4.0, tt[d][:, :],
                            op0=ALU.mult, op1=ALU.add,
                        )
                        nc.scalar.activation(sc[d][:, :], c2[d][:, :], AF.Sigmoid)
                        nc.vector.scalar_tensor_tensor(
                            hh[:, d, pos, :], sc[d][:, :], 0.5, sg[d][:, 3, :],
                            op0=ALU.subtract, op1=ALU.mult,
                        )
                assert gi == len(rest_groups)

            # ---- projection -> emissions -> E = exp(em + bp) ----
            with tc.tile_pool(name="pproj", bufs=2, space="PSUM") as pproj:
                for n in range(nch):
                    ps = pproj.tile([T, ncw], F32, tag="pp")
                    for d in range(2):
                        nc.tensor.matmul(
                            ps[:, :], lhsT=wp_sb[:, d, :],
                            rhs=hh[:, d].rearrange("p s b -> p (s b)")[:, n * ncw:(n + 1) * ncw],
                            start=(d == 0), stop=(d == 1),
                        )
                    nc.scalar.activation(
                        e_sb[0:T, n * ncw:(n + 1) * ncw], ps[:, :],
                        AF.Exp, bias=bp_sb[:, :], scale=1.0,
                    )

            # ---- CRF chunk-parallel scan: U <- diag(E_t) @ (P'^T U) ----
            with tc.tile_pool(name="pcrf", bufs=1, space="PSUM") as pcrf:
                for tau in range(CRF_SLOTS):
                    nact = sum(
                        1 for c in range(cc) if c * CRF_SLOTS + tau + 1 <= s - 1
                    )
                    if nact == 0:
                        continue
                    ps = pcrf.tile([T, 4, 512], F32, tag="pc")
                    c0 = 0
                    while c0 < nact:
                        cw = min(8, nact - c0)
                        nc.tensor.matmul(
                            ps[:, c0 // 8, 0:cw * 60], lhsT=pp_sb[:, :],
                            rhs=u_sb[:, c0 * 60:(c0 + cw) * 60],
                            start=True, stop=True,
                        )
                        c0 += cw
                    # fused E-scale update; PSUM banks hold 8 chunks (480
                    # of 512 cols) so reads are per-block strided
                    ebase = (tau + 1) * BL
                    full, remc = nact // 8, nact % 8
                    if full > 0:
                        eap = (
                            e_sb[0:T, ebase:ebase + 512 * full]
                            .rearrange("p (f c y) -> p f c y", f=full, c=8)
                            [:, :, :, 0:BL].unsqueeze(3)
                            .broadcast_to((T, full, 8, T, BL))
                        )
                        nc.vector.tensor_tensor(
                            u_sb[:, 0:full * 480]
                            .rearrange("p (f c m b) -> p f c m b", f=full, c=8, m=T),
                            ps[:, 0:full, 0:480]
                            .rearrange("p f (c m b) -> p f c m b", c=8, m=T),
                            eap, ALU.mult,
                        )
                    if remc > 0:
                        eap = (
                            e_sb[0:T, ebase + 512 * full:ebase + 512 * full + 64 * remc]
                            .rearrange("p (c y) -> p c y", c=remc)[:, :, 0:BL]
                            .unsqueeze(2)
                            .broadcast_to((T, remc, T, BL))
                        )
                        nc.vector.tensor_tensor(
                            u_sb[:, full * 480:full * 480 + remc * 60]
                            .rearrange("p (c m b) -> p c m b", c=remc, m=T),
                            ps[:, full, 0:remc * 60]
                            .rearrange("p (c m b) -> p c m b", c=remc, m=T),
                            eap, ALU.mult,
                        )

            # ---- emission-path gold score accumulators ----
            for d in range(2):
                scratch = sp.tile([128, sb], F32, tag="scr")
                nc.vector.scalar_tensor_tensor(
                    scratch[:, :],
                    hh[:, d].rearrange("p s b -> p (s b)"), 1.0, g_sb[:, d, :],
                    op0=ALU.mult, op1=ALU.mult,
                    accum_out=acc_sb[:, d:d + 1],
                )

            # ---- outputs ----
            nc.sync.dma_start(out=out_acc[:, :], in_=acc_sb[:, :])
            nc.sync.dma_start(out=out_u[:, :], in_=u_sb[:, :])
            nc.sync.dma_start(out=out_e0[:, :], in_=e_sb[0:T, 0:BL])
    return nc


# ---------------------------------------------------------------------------
# Host side
# ---------------------------------------------------------------------------

_NC_CACHE = {}


def _get_nc(s):
    if s not in _NC_CACHE:
        _NC_CACHE[s] = build_nc(s)
    return _NC_CACHE[s]


def kernel(x, tags, mask, Wih_f, Whh_f, bih_f, bhh_f, Wih_b, Whh_b, bih_b, bhh_b,
           Wp, bp, trans, start_t, end_t):
    x = np.asarray(x, np.float32)
    tags = np.asarray(tags)
    mask = np.asarray(mask)
    assert mask.all(), "kernel assumes mask == ones (spec fill: ones)"
    b, s, e = x.shape
    assert (b, e) == (B, E)
    cc = _crf_chunks(s)

    Wih = {0: np.asarray(Wih_f, np.float64), 1: np.asarray(Wih_b, np.float64)}
    Whh = {0: np.asarray(Whh_f, np.float64), 1: np.asarray(Whh_b, np.float64)}
    bias = {
        0: np.asarray(bih_f, np.float64) + np.asarray(bhh_f, np.float64),
        1: np.asarray(bih_b, np.float64) + np.asarray(bhh_b, np.float64),
    }
    Wp64 = np.asarray(Wp, np.float64)
    bp64 = np.asarray(bp, np.float64)
    trans64 = np.asarray(trans, np.float64)
    start64 = np.asarray(start_t, np.float64)
    end64 = np.asarray(end_t, np.float64)

    # gate folds: g-gate rows x2 (tanh via sigmoid); Whh/Wp x2 (h stored as h/2)
    gsl = slice(2 * H, 3 * H)
    wih_cols, whh_cols, bia_cols = [], [], []
    for d in range(2):
        wi = Wih[d].copy(); wi[gsl] *= 2.0
        wh = 2.0 * Whh[d].copy(); wh[gsl] *= 2.0
        bi = bias[d].copy(); bi[gsl] *= 2.0
        wih_cols.append(wi.T)        # (E, 4H)
        whh_cols.append(wh.T)        # (H, 4H)
        bia_cols.append(bi.reshape(4, H).T)   # (H, 4)
    wih_host = np.concatenate(wih_cols, axis=1).astype(bf16)       # (E, 8H)
    whh_host = np.concatenate(whh_cols, axis=1).astype(bf16)       # (H, 8H)
    bia_host = np.concatenate(bia_cols, axis=1).astype(np.float32)  # (H, 8)
    Wp_eff = 2.0 * Wp64                                             # (T, 2H)
    wpt_host = Wp_eff.T.astype(bf16)                                # (2H, T)
    bp_host = bp64.reshape(T, 1).astype(np.float32)
    pp_host = (np.exp(trans64) / 15.0).astype(bf16)           # (T, T)
    u0_host = np.zeros((T, cc * 60), bf16)
    for c in range(cc):
        for seq in range(BL):
            for m in range(T):
                u0_host[m, c * 60 + m * BL + seq] = 1.0

    in_maps = []
    for core in range(NCORES):
        bsl = slice(core * BL, (core + 1) * BL)
        xs = x[bsl]                                  # (BL, s, E)
        xt_host = np.ascontiguousarray(
            xs.transpose(2, 1, 0).reshape(E, s * BL)
        ).astype(bf16)                               # col = pos*BL + b
        tg = tags[bsl]                               # (BL, s)
        gm = np.empty((2 * H, s * BL), np.float64)
        gathered = Wp_eff[tg.T.reshape(-1)]          # (s*BL, 2H) rows pos-major
        gm[:, :] = gathered.T
        in_maps.append({
            "xt": xt_host,
            "wih": wih_host, "whh": whh_host, "bia": bia_host,
            "gmat": gm.astype(bf16), "wpt": wpt_host, "bp15": bp_host,
            "pp": pp_host, "u0": u0_host,
            "ident": np.eye(H, dtype=bf16),
        })

    nc = _get_nc(s)
    runner = globals()["run_bass_kernel_spmd"]
    if not getattr(runner, "_is_sim", False) and not getattr(nc, "_waits_split", False):
        _split_multi_waits(nc)
        nc._waits_split = True
    res = runner(nc, in_maps, core_ids=list(range(NCORES)))

    # ---- host epilogue ----
    logS = np.log(15.0)
    exp_start = np.exp(start64)
    exp_end = np.exp(end64)
    total = 0.0
    for core in range(NCORES):
        r = res.results[core]
        acc = np.asarray(r["out_acc"], np.float64)
        uu = np.asarray(r["out_u"], np.float64)
        e0 = np.asarray(r["out_e0"], np.float64)
        emscore = acc.sum()
        bsl = slice(core * BL, (core + 1) * BL)
        tg = tags[bsl]
        for seq in range(BL):
            tgq = tg[seq]
            score = (start64[tgq[0]] + trans64[tgq[:-1], tgq[1:]].sum()
                     + end64[tgq[-1]] + bp64[tgq].sum())
            rvec = exp_start * e0[:, seq]
            lz = 0.0
            for c in range(cc):
                ub = uu[:, c * 60 + seq:(c + 1) * 60:BL]   # (T, T): [j, m]
                rvec = ub @ rvec
                m = rvec.max()
                rvec /= m
                lz += np.log(m)
            lz += np.log(rvec @ exp_end) + (s - 1) * logS
            total += lz - score
        total -= emscore
    return np.asarray(total, np.float32)
